# revision 4
# baseline (speedup 1.0000x reference)
# Trainium2 Bass kernel for nn_CustomStyleLoss (segment-mean + MSE reduction).
#
# loss = sum_rows mean_chunks( (mean_chunk(input) - mean_chunk(style))^2 )
# with rows = 16*512 = 8192, each row = 50*50 = 2500 elems = 25 chunks of 100.
#
# Data-parallel over the row axis: core i gets rows [i*1024, (i+1)*1024).
# Per core: 4 tiles of [128 x 5000] f32 per tensor (2 rows per partition ->
# 20KB DMA lines for near-peak HBM bandwidth), input on the SP HWDGE ring,
# style on the ACT ring. One DVE tensor_tensor_scan computes the running sum
# of (input - style); chunk sums are strided differences of the scan output.
# ACT squares with the scale folded in (scale = 1/(100*sqrt(25)) = 0.002, so
# (0.002*chunk_sum)^2 sums directly to the loss) and accumulates the 50
# chunks into one column of a [128 x 4] partials tile. Host sums partials.

import sys

if "/opt/trn_rl_repo" not in sys.path:
    sys.path.insert(0, "/opt/trn_rl_repo")

import numpy as np

import concourse.bacc as bacc
import concourse.tile as tile
from concourse import mybir
from concourse.bass_utils import run_bass_kernel_spmd

N_CORES = 8
N_ROWS = 8192          # 16 * 512
K = 2500               # 50 * 50
CHUNK = 100
P = 128
ROWS_PER_PART = 2
F = K * ROWS_PER_PART               # 5000 elems / partition line
CPL = F // CHUNK                    # 50 chunks per partition line
ROWS_PER_CORE = N_ROWS // N_CORES   # 1024
ROWS_PER_TILE = P * ROWS_PER_PART   # 256
N_TILES = ROWS_PER_CORE // ROWS_PER_TILE  # 4
# (0.002 * chunk_sum)^2 == chunk_sum^2 / 100^2 / 25  ->  summing these over
# chunks/rows/cores gives the loss directly.
SCALE = 1.0 / (CHUNK * np.sqrt(K // CHUNK))

_CACHED_NC = None


def _build_nc():
    nc = bacc.Bacc(
        "TRN2",
        target_bir_lowering=False,
        debug=False,
        num_devices=N_CORES,
    )
    x = nc.dram_tensor(
        "input", [ROWS_PER_CORE, K], mybir.dt.float32, kind="ExternalInput"
    ).ap()
    s = nc.dram_tensor(
        "style", [ROWS_PER_CORE, K], mybir.dt.float32, kind="ExternalInput"
    ).ap()
    o = nc.dram_tensor(
        "out", [P, N_TILES], mybir.dt.float32, kind="ExternalOutput"
    ).ap()

    with tile.TileContext(nc) as tc:
        with (
            tc.tile_pool(name="io", bufs=3) as io_pool,
            tc.tile_pool(name="work", bufs=2) as work_pool,
            tc.tile_pool(name="stats", bufs=1) as stats_pool,
        ):
            partials = stats_pool.tile([P, N_TILES], mybir.dt.float32)
            for t in range(N_TILES):
                r0 = t * ROWS_PER_TILE
                xt = io_pool.tile([P, F], mybir.dt.float32, tag="xt")
                st = io_pool.tile([P, F], mybir.dt.float32, tag="st")
                x_src = x[r0 : r0 + ROWS_PER_TILE, :].rearrange(
                    "(p r) k -> p (r k)", r=ROWS_PER_PART
                )
                s_src = s[r0 : r0 + ROWS_PER_TILE, :].rearrange(
                    "(p r) k -> p (r k)", r=ROWS_PER_PART
                )
                # Both on the SP HWDGE ring: one ring keeps all 16 SDMA
                # engines busy, and strict FIFO order (in0,st0,in1,st1,...)
                # makes tile pairs complete evenly with no inter-queue skew.
                nc.sync.dma_start(out=xt, in_=x_src)
                nc.sync.dma_start(out=st, in_=s_src)

                # sc[:, j] = sum_{i<=j} (xt[:, i] - st[:, i])  (fp32 state)
                sc = work_pool.tile([P, F], mybir.dt.float32, tag="sc")
                nc.vector.tensor_tensor_scan(
                    out=sc,
                    data0=xt,
                    data1=st,
                    initial=0.0,
                    op0=mybir.AluOpType.add,
                    op1=mybir.AluOpType.subtract,
                )

                # chunk sums from scan boundaries: cs[c] = S[100c+99] - S[100c-1]
                hi = sc[:, CHUNK - 1 : F : CHUNK]      # [P, CPL]
                cs = work_pool.tile([P, CPL], mybir.dt.float32, tag="cs")
                # On GpSimd to keep the DVE free for the scans.
                nc.gpsimd.tensor_copy(cs[:, 0:1], hi[:, 0:1])
                nc.gpsimd.tensor_sub(cs[:, 1:CPL], hi[:, 1:CPL], hi[:, 0 : CPL - 1])

                # partials[:, t] = sum_c (SCALE * cs[:, c])^2
                sq = work_pool.tile([P, CPL], mybir.dt.float32, tag="sq")
                nc.scalar.activation(
                    out=sq,
                    in_=cs,
                    func=mybir.ActivationFunctionType.Square,
                    scale=float(SCALE),
                    accum_out=partials[:, t : t + 1],
                )
            nc.sync.dma_start(out=o, in_=partials)
    nc.compile()
    return nc


def _get_nc():
    global _CACHED_NC
    if _CACHED_NC is None:
        _CACHED_NC = _build_nc()
    return _CACHED_NC


def run_sharded(input, style, **run_kwargs):
    """Shard, run on 8 cores, return (scalar loss, BassKernelResults)."""
    nc = _get_nc()
    xi = np.ascontiguousarray(np.asarray(input, dtype=np.float32)).reshape(
        N_ROWS, K
    )
    xs = np.ascontiguousarray(np.asarray(style, dtype=np.float32)).reshape(
        N_ROWS, K
    )
    in_maps = [
        {
            "input": xi[i * ROWS_PER_CORE : (i + 1) * ROWS_PER_CORE],
            "style": xs[i * ROWS_PER_CORE : (i + 1) * ROWS_PER_CORE],
        }
        for i in range(N_CORES)
    ]
    res = run_bass_kernel_spmd(nc, in_maps, list(range(N_CORES)), **run_kwargs)
    total = np.float64(0.0)
    for r in res.results:
        total += r["out"].astype(np.float64).sum()
    return np.array(total, dtype=np.float32), res


def kernel(input, style):
    loss, _ = run_sharded(input, style)
    return loss
